# revision 1
# baseline (speedup 1.0000x reference)
"""Trainium2 Bass kernel for nn_Attention_3607772529228 (sparse_attention).

Reference computation (B=64, S=512, T=32, 2H=1024, ATT=512):
    ht_mean = mean(ht, axis=1)                               [B, 2H]
    z       = [h ; ht_mean] @ w1_w.T + w1_b                  [B, S, ATT]
    a       = tanh(z)
    beta    = a @ u_w[0];  beta = where(mask, beta, -1e20)   [B, S]
    alpha   = softmax(beta, axis=1)
    out     = einsum('bs,bsd->bd', alpha, h)                 [B, 2H]

Algebraic simplifications used (exact, not approximations):
  * The where(valid, ..., 0) maskings of h_cat and `a` in the reference do
    not affect the output: invalid positions only enter through beta, which
    is overwritten with -1e20 before the softmax.
  * The ht_mean half of the big matmul is constant over S, so it folds into
    a per-batch bias:  z = h @ w1.T + (w2 @ ht_mean + w1_b).

Distribution: data-parallel over batch B across 8 cores (8 batches/core).

Per-core layout (partition dim first):
  * z is computed as [ATT(part), S(free)] tiles:  lhsT = w1.T chunks
    (stationary), rhs = h.T chunks (moving, N=512).  h.T arrives via
    hardware DMA-transpose (bf16) straight from DRAM.
  * the per-batch bias lands on partitions -> added inside the ScalarE
    tanh (bias arg), fp32 exact.
  * beta = u . a via matmul with u columns stationary (M=1), 4 batches
    packed into distinct PE column groups (tile_position) to run
    concurrently.
  * softmax over the free dim on an [8, S] tile; alpha transposed with the
    PE; weighted sum alpha @ h uses natively-laid-out h (second bf16 copy),
    also column-group packed.
  * ~3.4us of warmup matmuls at kernel start bring the PE HAM clock gate
    to 2.4 GHz while the first DMAs are in flight.
"""

import os
from contextlib import ExitStack

import numpy as np
import ml_dtypes

import concourse.bass as bass
import concourse.tile as tile
from concourse import bacc, mybir
from concourse import bass_utils
from concourse.masks import make_identity

BF16 = mybir.dt.bfloat16
F32 = mybir.dt.float32

DEBUG_TAPS = False  # set True (before build) to add intermediate outputs

B, S, T, H2, ATT = 64, 512, 32, 1024, 512
NCORES = 8
BL = B // NCORES  # 8 batches per core
P = 128
KC = H2 // P  # 8 k-chunks over hidden
TT = ATT // P  # 4 attention tiles
SC = S // P  # 4 sequence chunks
NH = H2 // 512  # 2 output halves
NG = BL // 4  # batch groups of 4 (PE column-group packing)
WARMUP_MMS = 16
WSUM_DVE = False  # VectorE wsum: broken + slow on HW (sim-only correct); keep PE path


def _body(tc, reps=1):
    nc = tc.nc
    ctx = tc._ctx  # ExitStack stored by build()

    h_ap = nc.dram_tensor("h_bf", [BL, S, H2], BF16, kind="ExternalInput").ap()
    ht_ap = nc.dram_tensor("htt_bf", [H2, BL * T], BF16, kind="ExternalInput").ap()
    h_t_ap = nc.dram_tensor("h_t", [BL, H2, S], BF16, kind="ExternalInput").ap()
    w1t_ap = nc.dram_tensor("w1t", [H2, ATT], BF16, kind="ExternalInput").ap()
    w2t_ap = nc.dram_tensor("w2t", [H2, ATT], BF16, kind="ExternalInput").ap()
    u_ap = nc.dram_tensor("u_col", [P, TT, 32], BF16, kind="ExternalInput").ap()
    w1b_ap = nc.dram_tensor("w1b_col", [P, TT], F32, kind="ExternalInput").ap()
    mask_ap = nc.dram_tensor("maskadd", [BL, S], F32, kind="ExternalInput").ap()
    bsel_ap = nc.dram_tensor("bsel", [BL, BL * P], BF16, kind="ExternalInput").ap()
    out_ap = nc.dram_tensor("out", [BL, H2], F32, kind="ExternalOutput").ap()

    singles = ctx.enter_context(tc.tile_pool(name="singles", bufs=1))
    hT_pool = ctx.enter_context(
        tc.tile_pool(name="hT", bufs=(9 if WSUM_DVE else 2))
    )
    a_pool = ctx.enter_context(tc.tile_pool(name="a", bufs=20))
    rows = ctx.enter_context(tc.tile_pool(name="rows", bufs=4))
    z_psum = ctx.enter_context(
        tc.tile_pool(name="z_ps", bufs=(5 if WSUM_DVE else 3), space="PSUM")
    )
    b2_psum = ctx.enter_context(tc.tile_pool(name="b2_ps", bufs=1, space="PSUM"))
    beta_psum = ctx.enter_context(
        tc.tile_pool(name="beta_ps", bufs=(2 if WSUM_DVE else 1), space="PSUM")
    )
    if not WSUM_DVE:
        aT_psum = ctx.enter_context(tc.tile_pool(name="aT_ps", bufs=1, space="PSUM"))
        ws_psum = ctx.enter_context(tc.tile_pool(name="ws_ps", bufs=2, space="PSUM"))

    def emit():
        # ---- PE HAM warmup: keep TensorE busy while first DMAs land ----
        warm = singles.tile([P, S], BF16)
        nc.vector.memset(warm, 0.0)
        warm_ps = b2_psum.tile([P, S], F32, tag="b2")
        for _ in range(WARMUP_MMS):
            nc.tensor.matmul(
                warm_ps, lhsT=warm[:, 0:P], rhs=warm, start=True, stop=True
            )

        # ---- first batch's h loads, then weights ----
        hT_tiles = [None] * BL
        h_nat = None if WSUM_DVE else singles.tile([P, BL, SC, H2], BF16)

        def load_batch(b):
            hT_b = hT_pool.tile([P, KC, S], BF16, tag="hT")
            nc.scalar.dma_start(
                out=hT_b, in_=h_t_ap[b].rearrange("(k p) s -> p k s", p=P)
            )
            hT_tiles[b] = hT_b
            if not WSUM_DVE:
                nc.sync.dma_start(
                    out=h_nat[:, b, :, :],
                    in_=h_ap[b].rearrange("(sc p) d -> p sc d", p=P),
                )

        load_batch(0)
        w1t_sb = singles.tile([P, KC, ATT], BF16)
        nc.sync.dma_start(out=w1t_sb, in_=w1t_ap.rearrange("(k p) a -> p k a", p=P))

        # ---- constants / small inputs ----
        u_sb = singles.tile([P, TT, 32], BF16)
        nc.sync.dma_start(out=u_sb, in_=u_ap)
        w1b_sb = singles.tile([P, TT], F32)
        nc.sync.dma_start(out=w1b_sb, in_=w1b_ap)
        mask_sb = singles.tile([BL, S], F32)
        nc.sync.dma_start(out=mask_sb, in_=mask_ap)
        bsel_sb = singles.tile([BL, BL * P], BF16)
        nc.sync.dma_start(out=bsel_sb, in_=bsel_ap)
        ident = singles.tile([P, P], BF16)
        make_identity(nc, ident)
        w2t_sb = singles.tile([P, KC, ATT], BF16)
        nc.sync.dma_start(out=w2t_sb, in_=w2t_ap.rearrange("(k p) a -> p k a", p=P))

        # ---- ht mean -> per-batch bias columns ----
        htm = singles.tile([P, KC, BL], BF16)
        htT_sb = singles.tile([P, KC, BL * T], BF16)
        nc.scalar.dma_start(
            out=htT_sb, in_=ht_ap.rearrange("(c p) j -> p c j", p=P)
        )
        for c in range(KC):
            with nc.allow_low_precision("bf16 sum of 32 bf16 values, fp32 internal"):
                nc.vector.reduce_sum(
                    out=htm[:, c, :],
                    in_=htT_sb[:, c, :].rearrange("p (b t) -> p b t", b=BL),
                    axis=mybir.AxisListType.X,
                )

        # bias_col[att_tile] = (w2 @ ht_sum)/T + w1_b   ([128, BL] per tile)
        bias_col = singles.tile([P, TT, BL], F32)
        for t in range(TT):
            b2_ps = b2_psum.tile([P, S], F32, tag="b2")
            for c in range(KC):
                nc.tensor.matmul(
                    b2_ps[:, 0:BL],
                    lhsT=w2t_sb[:, c, t * P : (t + 1) * P],
                    rhs=htm[:, c, :],
                    start=(c == 0),
                    stop=(c == KC - 1),
                )
            nc.vector.tensor_scalar(
                out=bias_col[:, t, :],
                in0=b2_ps[:, 0:BL],
                scalar1=1.0 / T,
                scalar2=w1b_sb[:, t : t + 1],
                op0=mybir.AluOpType.mult,
                op1=mybir.AluOpType.add,
            )

        # ---- main pipeline: z matmul + tanh per batch; beta packed by 4 ----
        beta_all = singles.tile([BL, S], F32)
        a_tiles = {}
        for g in range(NG):
            for bb in range(4):
                b = 4 * g + bb
                if b + 1 < BL:
                    load_batch(b + 1)
                hT_b = hT_tiles[b]
                for t in range(TT):
                    z_ps = z_psum.tile([P, S], F32, tag="z")
                    for k in range(KC):
                        nc.tensor.matmul(
                            z_ps,
                            lhsT=w1t_sb[:, k, t * P : (t + 1) * P],
                            rhs=hT_b[:, k, :],
                            start=(k == 0),
                            stop=(k == KC - 1),
                        )
                    a_t = a_pool.tile([P, S], BF16, tag="a")
                    nc.scalar.activation(
                        out=a_t,
                        in_=z_ps,
                        func=mybir.ActivationFunctionType.Tanh,
                        bias=bias_col[:, t, b : b + 1],
                        scale=1.0,
                    )
                    a_tiles[(b, t)] = a_t
            # beta for the 4 batches of this group, one PE column group each
            beta_ps = beta_psum.tile([P, S], F32, tag="beta")
            for bb in range(4):
                b = 4 * g + bb
                for t in range(TT):
                    nc.tensor.matmul(
                        beta_ps[32 * bb : 32 * bb + 32, :],
                        lhsT=u_sb[:, t, :],
                        rhs=a_tiles[(b, t)],
                        start=(t == 0),
                        stop=(t == TT - 1),
                        tile_position=(0, 32 * bb),
                    )
            beta_sc = rows.tile([P, S], F32, tag="betarow")
            nc.scalar.copy(beta_sc, beta_ps)
            # strided gather: partitions {0,32,64,96} -> beta_all[4g:4g+4]
            nc.gpsimd.dma_start(
                out=beta_all[4 * g : 4 * g + 4, :],
                in_=beta_sc.rearrange("(b r) s -> b r s", r=32)[:, 0, :],
            )

        if DEBUG_TAPS:
            dbg_beta = nc.dram_tensor(
                "dbg_beta", [BL, S], F32, kind="ExternalOutput"
            ).ap()
            nc.gpsimd.dma_start(out=dbg_beta, in_=beta_all)
            dbg_bias = nc.dram_tensor(
                "dbg_bias", [P, TT, BL], F32, kind="ExternalOutput"
            ).ap()
            nc.gpsimd.dma_start(out=dbg_bias, in_=bias_col)

        # ---- softmax over S (free dim) for all 8 batches at once ----
        beta_m = singles.tile([BL, S], F32)
        nc.vector.tensor_add(beta_m, beta_all, mask_sb)
        negmax = singles.tile([BL, 1], F32)
        nc.vector.reduce_max(
            out=negmax, in_=beta_m, axis=mybir.AxisListType.X, negate=True
        )
        ex = singles.tile([BL, S], F32)
        sumrow = singles.tile([BL, 1], F32)
        nc.scalar.activation(
            out=ex,
            in_=beta_m,
            func=mybir.ActivationFunctionType.Exp,
            bias=negmax[:, 0:1],
            scale=1.0,
            accum_out=sumrow[:, 0:1],
        )
        rinv = singles.tile([BL, 1], F32)
        nc.vector.reciprocal(rinv, sumrow)
        alpha_bf = singles.tile([BL, S], BF16)
        nc.vector.tensor_scalar_mul(alpha_bf, ex, rinv[:, 0:1])

        if WSUM_DVE:
            # ---- weighted sum on VectorE: out[d] = sum_s hT[d,s]*alpha[s] ----
            s_cols = singles.tile([P, KC, BL], F32)
            prod = rows.tile([P, S], BF16, tag="prod")
            for b in range(BL):
                bc_ps = beta_psum.tile([P, S], F32, tag="beta")
                nc.tensor.matmul(
                    bc_ps,
                    lhsT=bsel_sb[:, b * P : (b + 1) * P],
                    rhs=alpha_bf,
                    start=True,
                    stop=True,
                )
                alpha_full = rows.tile([P, S], BF16, tag="afull")
                nc.scalar.copy(alpha_full, bc_ps)
                hT_b = hT_tiles[b]
                for c in range(KC):
                    nc.vector.scalar_tensor_tensor(
                        out=prod,
                        in0=hT_b[:, c, :],
                        scalar=1.0,
                        in1=alpha_full,
                        op0=mybir.AluOpType.mult,
                        op1=mybir.AluOpType.mult,
                        accum_out=s_cols[:, c, b : b + 1],
                    )
            # host un-permutes: out_perm[b, p, c] = s_cols[p, c, b]
            for b in range(BL):
                nc.gpsimd.dma_start(
                    out=out_ap[b].rearrange("(p c) -> p c", p=P),
                    in_=s_cols[:, :, b],
                )
        else:
            # ---- transpose alpha: [BL, S] -> 4x [128, BL] via PE ----
            alpha_rep = singles.tile([P, SC, BL, 32], BF16)
            for sc in range(SC):
                aT_ps = aT_psum.tile([P, BL], BF16, tag="aT")
                nc.tensor.transpose(
                    aT_ps,
                    alpha_bf[0:BL, sc * P : (sc + 1) * P],
                    ident[0:BL, 0:BL],
                )
                aT_bcast = bass.AP(
                    tensor=aT_ps.tensor,
                    offset=aT_ps.offset,
                    ap=[aT_ps.ap[0], aT_ps.ap[1], [0, 32]],
                )
                nc.vector.tensor_copy(out=alpha_rep[:, sc, :, :], in_=aT_bcast)

            # ---- weighted sum, 4 batches packed in PE column groups ----
            for g in range(NG):
                for nh in range(NH):
                    ws_ps = ws_psum.tile([P, 512], F32, tag="ws")
                    for bb in range(4):
                        b = 4 * g + bb
                        for sc in range(SC):
                            nc.tensor.matmul(
                                ws_ps[32 * bb : 32 * bb + 32, :],
                                lhsT=alpha_rep[:, sc, b, :],
                                rhs=h_nat[:, b, sc, nh * 512 : (nh + 1) * 512],
                                start=(sc == 0),
                                stop=(sc == SC - 1),
                                tile_position=(0, 32 * bb),
                            )
                    o_sc = rows.tile([P, 512], F32, tag="orow")
                    nc.scalar.copy(o_sc, ws_ps)
                    nc.gpsimd.dma_start(
                        out=out_ap[4 * g : 4 * g + 4, nh * 512 : (nh + 1) * 512],
                        in_=o_sc.rearrange("(b r) s -> b r s", r=32)[:, 0, :],
                    )

    for _rep in range(reps):
        emit()


_CACHE = {}


def build(reps=1):
    key = ("nc", reps)
    if key in _CACHE:
        return _CACHE[key]
    nc = bacc.Bacc("TRN2", target_bir_lowering=False, debug=False)
    with tile.TileContext(nc) as tc:
        with ExitStack() as ctx:
            tc._ctx = ctx
            _body(tc, reps=reps)
    nc.compile()
    _CACHE[key] = nc
    return nc


def _prep_core_inputs(h, h_mask, ht, w1_w, w1_b, u_w):
    """Host-side sharding + layout prep. Returns list of 8 per-core dicts."""
    bf = ml_dtypes.bfloat16
    h_bf = np.asarray(h, dtype=np.float32).astype(bf)
    ht_bf = np.asarray(ht, dtype=np.float32).astype(bf)
    w1t = np.ascontiguousarray(np.asarray(w1_w[:, :H2], dtype=np.float32).T).astype(bf)
    w2t = np.ascontiguousarray(np.asarray(w1_w[:, H2:], dtype=np.float32).T).astype(bf)
    u_col = np.ascontiguousarray(
        np.repeat(
            np.asarray(u_w[0], dtype=np.float32).reshape(TT, P).T[:, :, None],
            32,
            axis=2,
        )
    ).astype(bf)
    w1b_col = np.ascontiguousarray(
        np.asarray(w1_b, dtype=np.float32).reshape(TT, P).T
    ).astype(np.float32)
    maskadd = np.where(np.asarray(h_mask) != 0, 0.0, -1.0e20).astype(np.float32)
    bsel = np.zeros((BL, BL, P), dtype=np.float32)
    for b in range(BL):
        bsel[b, b, :] = 1.0
    bsel = bsel.reshape(BL, BL * P).astype(bf)

    in_maps = []
    for core in range(NCORES):
        lo, hi = core * BL, (core + 1) * BL
        in_maps.append(
            {
                "h_bf": np.ascontiguousarray(h_bf[lo:hi]),
                "h_t": np.ascontiguousarray(h_bf[lo:hi].transpose(0, 2, 1)),
                "htt_bf": np.ascontiguousarray(
                    ht_bf[lo:hi].reshape(BL * T, H2).T
                ),
                "w1t": w1t,
                "w2t": w2t,
                "u_col": u_col,
                "w1b_col": w1b_col,
                "maskadd": np.ascontiguousarray(maskadd[lo:hi]),
                "bsel": bsel,
            }
        )
    return in_maps


def kernel(h, h_mask, ht, w1_w, w1_b, u_w):
    nc = build()
    in_maps = _prep_core_inputs(h, h_mask, ht, w1_w, w1_b, u_w)
    res = bass_utils.run_bass_kernel_spmd(
        nc,
        in_maps,
        core_ids=list(range(NCORES)),
        trace=bool(int(os.environ.get("KERNEL_TRACE", "0"))),
    )
    _CACHE["last_result"] = res
    out = np.concatenate([r["out"] for r in res.results], axis=0)
    if WSUM_DVE:
        out = out.reshape(B, P, KC).transpose(0, 2, 1).reshape(B, H2)
    return np.ascontiguousarray(out.astype(np.float32))



# revision 7
# speedup vs baseline: 1.9590x; 1.9590x over previous
"""Trainium2 Bass kernel for nn_Attention_3607772529228 (sparse_attention).

Reference computation (B=64, S=512, T=32, 2H=1024, ATT=512):
    ht_mean = mean(ht, axis=1)                               [B, 2H]
    z       = [h ; ht_mean] @ w1_w.T + w1_b                  [B, S, ATT]
    a       = tanh(z)
    beta    = a @ u_w[0];  beta = where(mask, beta, -1e20)   [B, S]
    alpha   = softmax(beta, axis=1)
    out     = einsum('bs,bsd->bd', alpha, h)                 [B, 2H]

Algebraic simplifications (exact):
  * where(valid, ..., 0) on h_cat / a does not affect the output (invalid
    positions only enter through beta, overwritten with -1e20).
  * The ht_mean half of the big matmul folds into a per-batch bias:
    z = h @ w1.T + (w2 @ ht_mean + w1_b).

Distribution: data-parallel over batch B across 8 cores (8 batches/core).

v2 design (steady state is HBM-DMA-bound at ~19MB/core in the bf16
baseline; this version ships ~14MB and shortens the cold-run critical
path):
  * z path in fp8-e4m3: h_t, w1, w2 shipped fp8 (weights pre-scaled x64 to
    avoid the e4m3 subnormal range; folded back via the tanh activation
    scale). z matmuls use DoubleRow (2 fp8 weights/cell, K=256 per pass).
    Final rel err ~0.9e-2 (numpy-validated) vs 2e-2 budget.
  * h_nat (weighted-sum copy) stays bf16 -- output precision needs it.
  * All big DMAs are host-prepped to fully contiguous per-partition runs.
  * beta is computed into a 32x-replicated [128, S] layout (4 batches x 32
    replica partitions) and the softmax runs in that layout: no SWDGE
    gather DMAs, and the PE transpose of alpha directly yields the
    32-wide replicated columns the weighted-sum matmul needs.
  * Per-group tail pipelining: group 0's beta/softmax/wsum PE work is
    emitted between group 1's z matmuls, so only group 1's tail is
    exposed at the end.
  * PE warmup matmuls only on rep 0 (cold path).
"""

import os
from contextlib import ExitStack

import numpy as np
import ml_dtypes

import concourse.bass as bass
import concourse.tile as tile
from concourse import bacc, mybir
from concourse import bass_utils
from concourse.masks import make_identity

BF16 = mybir.dt.bfloat16
F8 = mybir.dt.float8e4
F32 = mybir.dt.float32

B, S, T, H2, ATT = 64, 512, 32, 1024, 512
NCORES = 8
BL = B // NCORES  # 8 batches per core
P = 128
KC = H2 // P  # 8 k-chunks over hidden
KD = KC // 2  # 4 DoubleRow k-pairs
TT = ATT // P  # 4 attention tiles
SC = S // P  # 4 sequence chunks
NH = H2 // 512  # 2 output halves
NG = BL // 4  # batch groups of 4 (PE column-group packing)
WARMUP_MMS = 12
FP8_Z = True  # fp8 z path (h_t/w1/w2) with DoubleRow matmuls
WS = 64.0 if FP8_Z else 1.0  # fp8 weight pre-scale (dodges e4m3 subnormals)

ZDT = F8 if FP8_Z else BF16
NP_F8 = ml_dtypes.float8_e4m3


def _body(tc, reps=1):
    nc = tc.nc
    ctx = tc._ctx

    h_ap = nc.dram_tensor("h_nat", [BL, P, SC * H2], BF16, kind="ExternalInput").ap()
    h8_ap = nc.dram_tensor("h8t", [BL, P, KC * S], ZDT, kind="ExternalInput").ap()
    w1_ap = nc.dram_tensor("w1t8", [P, KC * ATT], ZDT, kind="ExternalInput").ap()
    w2_ap = nc.dram_tensor("w2t8", [P, KC * ATT], ZDT, kind="ExternalInput").ap()
    htt_ap = nc.dram_tensor("htt_bf", [P, KC * BL * T], BF16, kind="ExternalInput").ap()
    u_ap = nc.dram_tensor("u_col", [P, TT * 32], BF16, kind="ExternalInput").ap()
    w1b_ap = nc.dram_tensor("w1b_col", [P, TT], F32, kind="ExternalInput").ap()
    mask_ap = nc.dram_tensor("maskrep", [P, NG * S], BF16, kind="ExternalInput").ap()
    out_ap = nc.dram_tensor("out", [BL, H2], F32, kind="ExternalOutput").ap()

    singles = ctx.enter_context(tc.tile_pool(name="singles", bufs=1))
    hT_pool = ctx.enter_context(tc.tile_pool(name="hT", bufs=3))
    a_pool = ctx.enter_context(tc.tile_pool(name="a", bufs=20))
    rows = ctx.enter_context(tc.tile_pool(name="rows", bufs=4))
    z_psum = ctx.enter_context(tc.tile_pool(name="z_ps", bufs=4, space="PSUM"))
    bias_ws_psum = ctx.enter_context(tc.tile_pool(name="bw_ps", bufs=2, space="PSUM"))
    beta_aT_psum = ctx.enter_context(tc.tile_pool(name="ba_ps", bufs=2, space="PSUM"))

    def emit(rep):
        # ---- PE HAM warmup (cold run only): busy while first DMAs land ----
        if rep == 0:
            warm = singles.tile([P, S], BF16)
            nc.vector.memset(warm, 0.0)
            warm_ps = bias_ws_psum.tile([P, S], F32, tag="bw")
            for _ in range(WARMUP_MMS):
                nc.tensor.matmul(
                    warm_ps, lhsT=warm[:, 0:P], rhs=warm, start=True, stop=True
                )

        # ---- weights + first batches; all h_nat upfront (sync queue) ----
        w1t_sb = singles.tile([P, KC, ATT], ZDT)
        nc.sync.dma_start(out=w1t_sb, in_=w1_ap.rearrange("p (k a) -> p k a", k=KC))

        hT_tiles = [None] * BL

        def load_hT(b):
            hT_b = hT_pool.tile([P, KC, S], ZDT, tag="hT")
            nc.scalar.dma_start(
                out=hT_b, in_=h8_ap[b].rearrange("p (k s) -> p k s", k=KC)
            )
            hT_tiles[b] = hT_b

        load_hT(0)
        load_hT(1)

        htt_sb = singles.tile([P, KC, BL * T], BF16)
        nc.scalar.dma_start(
            out=htt_sb, in_=htt_ap.rearrange("p (k j) -> p k j", k=KC)
        )

        h_nat = singles.tile([P, BL, SC, H2], BF16)
        for b in range(BL):
            nc.sync.dma_start(
                out=h_nat[:, b],
                in_=h_ap[b].rearrange("p (sc d) -> p sc d", sc=SC),
            )

        w2t_sb = singles.tile([P, KC, ATT], ZDT)
        nc.sync.dma_start(out=w2t_sb, in_=w2_ap.rearrange("p (k a) -> p k a", k=KC))
        u_sb = singles.tile([P, TT, 32], BF16)
        nc.sync.dma_start(out=u_sb, in_=u_ap.rearrange("p (t r) -> p t r", t=TT))
        w1b_sb = singles.tile([P, TT], F32)
        nc.sync.dma_start(out=w1b_sb, in_=w1b_ap)
        mask_sb = singles.tile([P, NG, S], BF16)
        nc.sync.dma_start(out=mask_sb, in_=mask_ap.rearrange("p (g s) -> p g s", g=NG))
        ident = singles.tile([P, P], BF16)
        make_identity(nc, ident)

        # ---- ht sum -> fp8 columns for the bias matmul ----
        htm = singles.tile([P, KC, BL], BF16)
        for c in range(KC):
            with nc.allow_low_precision("bf16 sum of 32 bf16 values, fp32 internal"):
                nc.vector.reduce_sum(
                    out=htm[:, c, :],
                    in_=htt_sb[:, c, :].rearrange("p (b t) -> p b t", b=BL),
                    axis=mybir.AxisListType.X,
                )
        if FP8_Z:
            htm_z = singles.tile([P, KC, BL], ZDT)
            nc.vector.tensor_copy(out=htm_z, in_=htm)
        else:
            htm_z = htm

        bias_col = singles.tile([P, TT, BL], F32)

        def emit_bias():
            # bias_col[t] = (w2*WS @ ht_sum)/(T*WS) + w1_b   ([128, BL] per t)
            for t in range(TT):
                b2_ps = bias_ws_psum.tile([P, S], F32, tag="bw")
                for c in range(KC):
                    nc.tensor.matmul(
                        b2_ps[:, 0:BL],
                        lhsT=w2t_sb[:, c, t * P : (t + 1) * P],
                        rhs=htm_z[:, c, :],
                        start=(c == 0),
                        stop=(c == KC - 1),
                    )
                nc.vector.tensor_scalar(
                    out=bias_col[:, t, :],
                    in0=b2_ps[:, 0:BL],
                    scalar1=1.0 / (T * WS),
                    scalar2=w1b_sb[:, t : t + 1],
                    op0=mybir.AluOpType.mult,
                    op1=mybir.AluOpType.add,
                )

        # ---- per-group tail: beta + softmax + wsum (replicated layout) ----
        a_tiles = {}
        beta_tiles = {}

        def emit_beta(g):
            # beta for 4 batches, batch 4g+j on partitions 32j..32j+31 (x32)
            beta_ps = beta_aT_psum.tile([P, S], F32, tag="ba")
            for bb in range(4):
                b = 4 * g + bb
                for t in range(TT):
                    nc.tensor.matmul(
                        beta_ps[32 * bb : 32 * bb + 32, :],
                        lhsT=u_sb[:, t, :],
                        rhs=a_tiles[(b, t)],
                        start=(t == 0),
                        stop=(t == TT - 1),
                        tile_position=(0, 32 * bb),
                    )
            beta_tiles[g] = beta_ps

        def emit_tail(g):
            # softmax over S (free dim), all 4 batches (x32 replicas) at once
            beta_ps = beta_tiles[g]
            beta_m = rows.tile([P, S], F32, tag="betam")
            nc.vector.tensor_add(beta_m, beta_ps, mask_sb[:, g, :])
            negmax = rows.tile([P, 1], F32, tag="negmax")
            nc.vector.reduce_max(
                out=negmax, in_=beta_m, axis=mybir.AxisListType.X, negate=True
            )
            ex = rows.tile([P, S], F32, tag="ex")
            sumrow = rows.tile([P, 1], F32, tag="sumrow")
            nc.scalar.activation(
                out=ex,
                in_=beta_m,
                func=mybir.ActivationFunctionType.Exp,
                bias=negmax[:, 0:1],
                scale=1.0,
                accum_out=sumrow[:, 0:1],
            )
            rinv = rows.tile([P, 1], F32, tag="rinv")
            nc.vector.reciprocal(rinv, sumrow)
            alpha_bf = rows.tile([P, S], BF16, tag="alpha")
            nc.vector.tensor_scalar_mul(alpha_bf, ex, rinv[:, 0:1])

            # PE transpose: [128(4bx32r), S] -> per sc [128(s), 128(4bx32r)]
            alpha_sb = rows.tile([P, SC, P], BF16, tag="alphasb")
            for sc in range(SC):
                aT_ps = beta_aT_psum.tile([P, P], BF16, tag="ba")
                nc.tensor.transpose(
                    aT_ps, alpha_bf[:, sc * P : (sc + 1) * P], ident
                )
                nc.scalar.copy(alpha_sb[:, sc, :], aT_ps)

            # weighted sum, 4 batches packed in PE column groups
            for nh in range(NH):
                ws_ps = bias_ws_psum.tile([P, 512], F32, tag="bw")
                for bb in range(4):
                    b = 4 * g + bb
                    for sc in range(SC):
                        nc.tensor.matmul(
                            ws_ps[32 * bb : 32 * bb + 32, :],
                            lhsT=alpha_sb[:, sc, 32 * bb : 32 * bb + 32],
                            rhs=h_nat[:, b, sc, nh * 512 : (nh + 1) * 512],
                            start=(sc == 0),
                            stop=(sc == SC - 1),
                            tile_position=(0, 32 * bb),
                        )
                o_sc = rows.tile([P, 512], F32, tag="orow")
                nc.scalar.copy(o_sc, ws_ps)
                # strided gather: partitions {0,32,64,96} -> out rows
                nc.gpsimd.dma_start(
                    out=out_ap[4 * g : 4 * g + 4, nh * 512 : (nh + 1) * 512],
                    in_=o_sc.rearrange("(b r) s -> b r s", r=32)[:, 0, :],
                )

        # ---- main pipeline: z matmul + tanh per batch ----
        def emit_z(b, t):
            z_ps = z_psum.tile([P, S], F32, tag="z")
            hT_b = hT_tiles[b]
            if FP8_Z:
                for kk in range(KD):
                    nc.tensor.matmul(
                        z_ps,
                        lhsT=w1t_sb[:, 2 * kk : 2 * kk + 2, t * P : (t + 1) * P],
                        rhs=hT_b[:, 2 * kk : 2 * kk + 2, :],
                        start=(kk == 0),
                        stop=(kk == KD - 1),
                        perf_mode=mybir.MatmulPerfMode.DoubleRow,
                    )
            else:
                for k in range(KC):
                    nc.tensor.matmul(
                        z_ps,
                        lhsT=w1t_sb[:, k, t * P : (t + 1) * P],
                        rhs=hT_b[:, k, :],
                        start=(k == 0),
                        stop=(k == KC - 1),
                    )
            return z_ps

        def emit_tanh(b, t, z_ps):
            a_t = a_pool.tile([P, S], BF16, tag="a")
            nc.scalar.activation(
                out=a_t,
                in_=z_ps,
                func=mybir.ActivationFunctionType.Tanh,
                bias=bias_col[:, t, b : b + 1],
                scale=1.0 / WS,
            )
            a_tiles[(b, t)] = a_t

        for b in range(BL):
            if b + 2 < BL:
                load_hT(b + 2)
            if b == 0:
                # bias MMs go on the PE queue between z(b0) and tanh(b0):
                # htm is ready by then, and tanh(b0) needs bias_col written
                z_tiles0 = [emit_z(0, t) for t in range(TT)]
                emit_bias()
                for t in range(TT):
                    emit_tanh(0, t, z_tiles0[t])
                continue
            for t in range(TT):
                emit_tanh(b, t, emit_z(b, t))
            if b == 4:
                emit_beta(0)
            elif b == 5:
                emit_tail(0)
        emit_beta(1)
        emit_tail(1)

    for rep in range(reps):
        emit(rep)


_CACHE = {}


def build(reps=1):
    key = ("nc", reps)
    if key in _CACHE:
        return _CACHE[key]
    nc = bacc.Bacc("TRN2", target_bir_lowering=False, debug=False)
    with tile.TileContext(nc) as tc:
        with ExitStack() as ctx:
            tc._ctx = ctx
            _body(tc, reps=reps)
    nc.compile()
    _CACHE[key] = nc
    return nc


def _prep_core_inputs(h, h_mask, ht, w1_w, w1_b, u_w):
    """Host-side sharding + layout prep. Returns list of 8 per-core dicts."""
    bf = ml_dtypes.bfloat16
    zdt = NP_F8 if FP8_Z else bf
    h = np.asarray(h, dtype=np.float32)
    ht = np.asarray(ht, dtype=np.float32)

    # h_nat[b, p, sc*H2 + d] = h[b, sc*128+p, d]   (contiguous per partition)
    h_nat = np.ascontiguousarray(
        h.reshape(B, SC, P, H2).transpose(0, 2, 1, 3).reshape(B, P, SC * H2)
    ).astype(bf)
    # h8t[b, p, k*S + s] = h[b, s, k*128+p]
    h8t = np.ascontiguousarray(
        h.transpose(0, 2, 1).reshape(B, KC, P, S).transpose(0, 2, 1, 3)
        .reshape(B, P, KC * S)
    ).astype(zdt)

    def prep_w(w):  # [ATT, H2] -> [P, KC*ATT]: w8[p, k*ATT+a] = w[a, k*128+p]
        wt = np.ascontiguousarray(np.asarray(w, dtype=np.float32).T)  # [H2, ATT]
        return np.ascontiguousarray(
            (wt * WS).reshape(KC, P, ATT).transpose(1, 0, 2).reshape(P, KC * ATT)
        ).astype(zdt)

    w1t8 = prep_w(w1_w[:, :H2])
    w2t8 = prep_w(w1_w[:, H2:])

    # htt[p, k*BL*T + j] = ht_flat[j, k*128+p]  (j = b*T + t), per core below
    u_col = np.ascontiguousarray(
        np.repeat(
            np.asarray(u_w[0], dtype=np.float32).reshape(TT, P).T[:, :, None],
            32,
            axis=2,
        ).reshape(P, TT * 32)
    ).astype(bf)
    w1b_col = np.ascontiguousarray(
        np.asarray(w1_b, dtype=np.float32).reshape(TT, P).T
    ).astype(np.float32)

    neg = np.float32(-1e20)
    maskadd = np.where(np.asarray(h_mask) != 0, np.float32(0.0), neg)  # [B, S]

    in_maps = []
    for core in range(NCORES):
        lo, hi = core * BL, (core + 1) * BL
        htc = ht[lo:hi].reshape(BL * T, H2).T  # [H2, BL*T]
        htt = np.ascontiguousarray(
            htc.reshape(KC, P, BL * T).transpose(1, 0, 2).reshape(P, KC * BL * T)
        ).astype(bf)
        # mrep[32*j+r, g*S+s] = maskadd[lo + 4g+j, s]
        mrep = np.ascontiguousarray(
            np.repeat(maskadd[lo:hi].reshape(NG, 4, 1, S), 32, axis=2)
            .reshape(NG, P, S).transpose(1, 0, 2).reshape(P, NG * S)
        ).astype(bf)
        in_maps.append(
            {
                "h_nat": np.ascontiguousarray(h_nat[lo:hi]),
                "h8t": np.ascontiguousarray(h8t[lo:hi]),
                "w1t8": w1t8,
                "w2t8": w2t8,
                "htt_bf": htt,
                "u_col": u_col,
                "w1b_col": w1b_col,
                "maskrep": np.ascontiguousarray(mrep),
            }
        )
    return in_maps


def kernel(h, h_mask, ht, w1_w, w1_b, u_w):
    nc = build()
    in_maps = _prep_core_inputs(h, h_mask, ht, w1_w, w1_b, u_w)
    res = bass_utils.run_bass_kernel_spmd(
        nc,
        in_maps,
        core_ids=list(range(NCORES)),
        trace=bool(int(os.environ.get("KERNEL_TRACE", "0"))),
    )
    _CACHE["last_result"] = res
    out = np.concatenate([r["out"] for r in res.results], axis=0)
    return np.ascontiguousarray(out.astype(np.float32))


# revision 8
# speedup vs baseline: 2.1494x; 1.0972x over previous
"""Trainium2 Bass kernel for nn_Attention_3607772529228 (sparse_attention).

Reference computation (B=64, S=512, T=32, 2H=1024, ATT=512):
    ht_mean = mean(ht, axis=1)                               [B, 2H]
    z       = [h ; ht_mean] @ w1_w.T + w1_b                  [B, S, ATT]
    a       = tanh(z)
    beta    = a @ u_w[0];  beta = where(mask, beta, -1e20)   [B, S]
    alpha   = softmax(beta, axis=1)
    out     = einsum('bs,bsd->bd', alpha, h)                 [B, 2H]

Algebraic simplifications (exact):
  * where(valid, ..., 0) on h_cat / a does not affect the output (invalid
    positions only enter through beta, overwritten with -1e20).
  * The ht_mean half of the big matmul folds into a per-batch bias:
    z = h @ w1.T + (w2 @ ht_mean + w1_b).

Distribution: data-parallel over batch B across 8 cores (8 batches/core).

v3 design (the bf16 baseline is HBM-DMA-bound at ~19MB/core/rep):
  * z path in fp8-e4m3: h_t, w1, w2 shipped fp8 (weights pre-scaled x64 to
    dodge the e4m3 subnormal range; folded back via the tanh activation
    scale). z matmuls use DoubleRow (2 fp8 weights/cell, K=256/pass).
    Final rel err ~0.9e-2 (validated vs 2e-2 budget).
  * h_nat (weighted-sum copy) stays bf16 -- output precision needs it.
  * All big DMAs are host-prepped fully contiguous per partition.
  * Constants (w1/w2/htt/u/mask) + the bias matmuls are hoisted out of
    the rep loop and issued before the h_nat stream.
  * beta lands in a 32x-replicated [128, S] layout (4 batches x 32
    replicas); softmax runs in that layout (no gather DMAs). The -1e20
    mask add is one extra identity-lhsT matmul accumulated into beta's
    PSUM. Exp writes bf16 directly (f32 row-sum via accum_out); the
    1/sum normalization is deferred to the output copy (per-partition
    scale), keeping the beta->alpha->transpose chain short.
  * Per-group tail pipelining: group 0's beta/softmax/wsum PE work is
    emitted between group 1's z matmuls; only group 1's tail is exposed.
  * PE warmup matmuls only on rep 0 (cold path).
"""

import os
from contextlib import ExitStack

import numpy as np
import ml_dtypes

import concourse.bass as bass
import concourse.tile as tile
from concourse import bacc, mybir
from concourse import bass_utils
from concourse.masks import make_identity

BF16 = mybir.dt.bfloat16
F8 = mybir.dt.float8e4
F32 = mybir.dt.float32

B, S, T, H2, ATT = 64, 512, 32, 1024, 512
NCORES = 8
BL = B // NCORES  # 8 batches per core
P = 128
KC = H2 // P  # 8 k-chunks over hidden
KD = KC // 2  # 4 DoubleRow k-pairs
TT = ATT // P  # 4 attention tiles
SC = S // P  # 4 sequence chunks
NH = H2 // 512  # 2 output halves
NG = BL // 4  # batch groups of 4 (PE column-group packing)
WARMUP_MMS = 12
FP8_Z = True  # fp8 z path (h_t/w1/w2) with DoubleRow matmuls
WS = 64.0 if FP8_Z else 1.0  # fp8 weight pre-scale (dodges e4m3 subnormals)

ZDT = F8 if FP8_Z else BF16
NP_F8 = ml_dtypes.float8_e4m3


def _body(tc, reps=1):
    nc = tc.nc
    ctx = tc._ctx

    h_ap = nc.dram_tensor("h_nat", [BL, P, SC * H2], BF16, kind="ExternalInput").ap()
    h8_ap = nc.dram_tensor("h8t", [BL, P, KC * S], ZDT, kind="ExternalInput").ap()
    w1_ap = nc.dram_tensor("w1t8", [P, KC * ATT], ZDT, kind="ExternalInput").ap()
    w2_ap = nc.dram_tensor("w2t8", [P, KC * ATT], ZDT, kind="ExternalInput").ap()
    htt_ap = nc.dram_tensor("htt_bf", [P, KC * BL * T], BF16, kind="ExternalInput").ap()
    u_ap = nc.dram_tensor("u_col", [P, TT * 32], BF16, kind="ExternalInput").ap()
    w1b_ap = nc.dram_tensor("w1b_col", [P, TT], F32, kind="ExternalInput").ap()
    mask_ap = nc.dram_tensor("maskrep", [P, NG * S], BF16, kind="ExternalInput").ap()
    out_ap = nc.dram_tensor("out", [BL, H2], F32, kind="ExternalOutput").ap()

    singles = ctx.enter_context(tc.tile_pool(name="singles", bufs=1))
    hT_pool = ctx.enter_context(tc.tile_pool(name="hT", bufs=3))
    a_pool = ctx.enter_context(tc.tile_pool(name="a", bufs=20))
    rows = ctx.enter_context(tc.tile_pool(name="rows", bufs=4))
    z_psum = ctx.enter_context(tc.tile_pool(name="z_ps", bufs=4, space="PSUM"))
    bias_ws_psum = ctx.enter_context(tc.tile_pool(name="bw_ps", bufs=2, space="PSUM"))
    beta_aT_psum = ctx.enter_context(tc.tile_pool(name="ba_ps", bufs=2, space="PSUM"))

    # ================= prologue: warmup, constants, bias =================
    warm = singles.tile([P, S], BF16)
    nc.vector.memset(warm, 0.0)
    warm_ps = bias_ws_psum.tile([P, S], F32, tag="bw")
    for _ in range(WARMUP_MMS):
        nc.tensor.matmul(warm_ps, lhsT=warm[:, 0:P], rhs=warm, start=True, stop=True)

    # scalar HWDGE queue: htt first (bias path), then the per-rep hT stream
    htt_sb = singles.tile([P, KC, BL * T], BF16)
    nc.scalar.dma_start(out=htt_sb, in_=htt_ap.rearrange("p (k j) -> p k j", k=KC))

    # sync HWDGE queue: weights + small constants, ahead of the h_nat stream
    w1t_sb = singles.tile([P, KC, ATT], ZDT)
    nc.sync.dma_start(out=w1t_sb, in_=w1_ap.rearrange("p (k a) -> p k a", k=KC))
    w2t_sb = singles.tile([P, KC, ATT], ZDT)
    nc.sync.dma_start(out=w2t_sb, in_=w2_ap.rearrange("p (k a) -> p k a", k=KC))
    u_sb = singles.tile([P, TT, 32], BF16)
    nc.sync.dma_start(out=u_sb, in_=u_ap.rearrange("p (t r) -> p t r", t=TT))
    w1b_sb = singles.tile([P, TT], F32)
    nc.sync.dma_start(out=w1b_sb, in_=w1b_ap)
    mask_sb = singles.tile([P, NG, S], BF16)
    nc.sync.dma_start(out=mask_sb, in_=mask_ap.rearrange("p (g s) -> p g s", g=NG))
    ident = singles.tile([P, P], BF16)
    make_identity(nc, ident)

    # ht sum -> (fp8) columns; bias_col[t] = (w2*WS @ ht_sum)/(T*WS) + w1_b
    htm = singles.tile([P, KC, BL], BF16)
    for c in range(KC):
        with nc.allow_low_precision("bf16 sum of 32 bf16 values, fp32 internal"):
            nc.vector.reduce_sum(
                out=htm[:, c, :],
                in_=htt_sb[:, c, :].rearrange("p (b t) -> p b t", b=BL),
                axis=mybir.AxisListType.X,
            )
    if FP8_Z:
        htm_z = singles.tile([P, KC, BL], ZDT)
        nc.vector.tensor_copy(out=htm_z, in_=htm)
    else:
        htm_z = htm
    bias_col = singles.tile([P, TT, BL], F32)
    for t in range(TT):
        b2_ps = bias_ws_psum.tile([P, S], F32, tag="bw")
        for c in range(KC):
            nc.tensor.matmul(
                b2_ps[:, 0:BL],
                lhsT=w2t_sb[:, c, t * P : (t + 1) * P],
                rhs=htm_z[:, c, :],
                start=(c == 0),
                stop=(c == KC - 1),
            )
        nc.vector.tensor_scalar(
            out=bias_col[:, t, :],
            in0=b2_ps[:, 0:BL],
            scalar1=1.0 / (T * WS),
            scalar2=w1b_sb[:, t : t + 1],
            op0=mybir.AluOpType.mult,
            op1=mybir.AluOpType.add,
        )

    # ========================== per-rep body ==========================
    def emit_rep():
        hT_tiles = [None] * BL
        a_tiles = {}
        beta_tiles = {}

        def load_hT(b):
            hT_b = hT_pool.tile([P, KC, S], ZDT, tag="hT")
            nc.scalar.dma_start(
                out=hT_b, in_=h8_ap[b].rearrange("p (k s) -> p k s", k=KC)
            )
            hT_tiles[b] = hT_b

        load_hT(0)
        load_hT(1)

        h_nat = singles.tile([P, BL, SC, H2], BF16)
        for half in range(2):
            nc.sync.dma_start(
                out=h_nat[:, 4 * half : 4 * half + 4],
                in_=h_ap[4 * half : 4 * half + 4].rearrange(
                    "b p (sc d) -> p b sc d", sc=SC
                ),
            )

        def emit_beta(g):
            # beta for 4 batches, batch 4g+j on partitions 32j..32j+31 (x32)
            beta_ps = beta_aT_psum.tile([P, S], F32, tag="ba")
            for bb in range(4):
                b = 4 * g + bb
                for t in range(TT):
                    nc.tensor.matmul(
                        beta_ps[32 * bb : 32 * bb + 32, :],
                        lhsT=u_sb[:, t, :],
                        rhs=a_tiles[(b, t)],
                        start=(t == 0),
                        stop=(t == TT - 1),
                        tile_position=(0, 32 * bb),
                    )
            # + mask (0 / -1e20, replicated layout) via identity-lhsT matmul
            nc.tensor.matmul(
                beta_ps,
                lhsT=ident,
                rhs=mask_sb[:, g, :],
                start=False,
                stop=True,
                skip_group_check=True,
            )
            beta_tiles[g] = beta_ps

        def emit_tail(g):
            # softmax over S (free dim), all 4 batches (x32 replicas) at once
            beta_ps = beta_tiles[g]
            negmax = rows.tile([P, 1], F32, tag="negmax")
            nc.vector.reduce_max(
                out=negmax, in_=beta_ps, axis=mybir.AxisListType.X, negate=True
            )
            alpha_bf = rows.tile([P, S], BF16, tag="alpha")
            sumrow = rows.tile([P, 1], F32, tag="sumrow")
            nc.scalar.activation(
                out=alpha_bf,
                in_=beta_ps,
                func=mybir.ActivationFunctionType.Exp,
                bias=negmax[:, 0:1],
                scale=1.0,
                accum_out=sumrow[:, 0:1],
            )
            rinv = rows.tile([P, 1], F32, tag="rinv")
            nc.vector.reciprocal(rinv, sumrow)

            # PE transpose: [128(4bx32r), S] -> per sc [128(s), 128(4bx32r)]
            alpha_sb = rows.tile([P, SC, P], BF16, tag="alphasb")
            for sc in range(SC):
                aT_ps = beta_aT_psum.tile([P, P], BF16, tag="ba")
                nc.tensor.transpose(
                    aT_ps, alpha_bf[:, sc * P : (sc + 1) * P], ident
                )
                nc.scalar.copy(alpha_sb[:, sc, :], aT_ps)

            # weighted sum (unnormalized), 4 batches in PE column groups;
            # normalization folds into the output copy as per-partition scale
            for nh in range(NH):
                ws_ps = bias_ws_psum.tile([P, 512], F32, tag="bw")
                for bb in range(4):
                    b = 4 * g + bb
                    for sc in range(SC):
                        nc.tensor.matmul(
                            ws_ps[32 * bb : 32 * bb + 32, :],
                            lhsT=alpha_sb[:, sc, 32 * bb : 32 * bb + 32],
                            rhs=h_nat[:, b, sc, nh * 512 : (nh + 1) * 512],
                            start=(sc == 0),
                            stop=(sc == SC - 1),
                            tile_position=(0, 32 * bb),
                        )
                o_sc = rows.tile([P, 512], F32, tag="orow")
                nc.scalar.activation(
                    out=o_sc,
                    in_=ws_ps,
                    func=mybir.ActivationFunctionType.Copy,
                    scale=rinv[:, 0:1],
                )
                # strided gather: partitions {0,32,64,96} -> out rows
                nc.gpsimd.dma_start(
                    out=out_ap[4 * g : 4 * g + 4, nh * 512 : (nh + 1) * 512],
                    in_=o_sc.rearrange("(b r) s -> b r s", r=32)[:, 0, :],
                )

        def emit_z(b, t):
            z_ps = z_psum.tile([P, S], F32, tag="z")
            hT_b = hT_tiles[b]
            if FP8_Z:
                for kk in range(KD):
                    nc.tensor.matmul(
                        z_ps,
                        lhsT=w1t_sb[:, 2 * kk : 2 * kk + 2, t * P : (t + 1) * P],
                        rhs=hT_b[:, 2 * kk : 2 * kk + 2, :],
                        start=(kk == 0),
                        stop=(kk == KD - 1),
                        perf_mode=mybir.MatmulPerfMode.DoubleRow,
                    )
            else:
                for k in range(KC):
                    nc.tensor.matmul(
                        z_ps,
                        lhsT=w1t_sb[:, k, t * P : (t + 1) * P],
                        rhs=hT_b[:, k, :],
                        start=(k == 0),
                        stop=(k == KC - 1),
                    )
            return z_ps

        for b in range(BL):
            if b + 2 < BL:
                load_hT(b + 2)
            for t in range(TT):
                z_ps = emit_z(b, t)
                a_t = a_pool.tile([P, S], BF16, tag="a")
                nc.scalar.activation(
                    out=a_t,
                    in_=z_ps,
                    func=mybir.ActivationFunctionType.Tanh,
                    bias=bias_col[:, t, b : b + 1],
                    scale=1.0 / WS,
                )
                a_tiles[(b, t)] = a_t
            if b == 4:
                emit_beta(0)
            elif b == 5:
                emit_tail(0)
        emit_beta(1)
        emit_tail(1)

    for _rep in range(reps):
        emit_rep()


_CACHE = {}


def build(reps=1):
    key = ("nc", reps)
    if key in _CACHE:
        return _CACHE[key]
    nc = bacc.Bacc("TRN2", target_bir_lowering=False, debug=False)
    with tile.TileContext(nc) as tc:
        with ExitStack() as ctx:
            tc._ctx = ctx
            _body(tc, reps=reps)
    nc.compile()
    _CACHE[key] = nc
    return nc


def _prep_core_inputs(h, h_mask, ht, w1_w, w1_b, u_w):
    """Host-side sharding + layout prep. Returns list of 8 per-core dicts."""
    bf = ml_dtypes.bfloat16
    zdt = NP_F8 if FP8_Z else bf
    h = np.asarray(h, dtype=np.float32)
    ht = np.asarray(ht, dtype=np.float32)

    # h_nat[b, p, sc*H2 + d] = h[b, sc*128+p, d]   (contiguous per partition)
    h_nat = np.ascontiguousarray(
        h.reshape(B, SC, P, H2).transpose(0, 2, 1, 3).reshape(B, P, SC * H2)
    ).astype(bf)
    # h8t[b, p, k*S + s] = h[b, s, k*128+p]
    h8t = np.ascontiguousarray(
        h.transpose(0, 2, 1).reshape(B, KC, P, S).transpose(0, 2, 1, 3)
        .reshape(B, P, KC * S)
    ).astype(zdt)

    def prep_w(w):  # [ATT, H2] -> [P, KC*ATT]: w8[p, k*ATT+a] = w[a, k*128+p]
        wt = np.ascontiguousarray(np.asarray(w, dtype=np.float32).T)  # [H2, ATT]
        return np.ascontiguousarray(
            (wt * WS).reshape(KC, P, ATT).transpose(1, 0, 2).reshape(P, KC * ATT)
        ).astype(zdt)

    w1t8 = prep_w(w1_w[:, :H2])
    w2t8 = prep_w(w1_w[:, H2:])

    u_col = np.ascontiguousarray(
        np.repeat(
            np.asarray(u_w[0], dtype=np.float32).reshape(TT, P).T[:, :, None],
            32,
            axis=2,
        ).reshape(P, TT * 32)
    ).astype(bf)
    w1b_col = np.ascontiguousarray(
        np.asarray(w1_b, dtype=np.float32).reshape(TT, P).T
    ).astype(np.float32)

    neg = np.float32(-1e20)
    maskadd = np.where(np.asarray(h_mask) != 0, np.float32(0.0), neg)  # [B, S]

    in_maps = []
    for core in range(NCORES):
        lo, hi = core * BL, (core + 1) * BL
        htc = ht[lo:hi].reshape(BL * T, H2).T  # [H2, BL*T]
        htt = np.ascontiguousarray(
            htc.reshape(KC, P, BL * T).transpose(1, 0, 2).reshape(P, KC * BL * T)
        ).astype(bf)
        # mrep[32*j+r, g*S+s] = maskadd[lo + 4g+j, s]
        mrep = np.ascontiguousarray(
            np.repeat(maskadd[lo:hi].reshape(NG, 4, 1, S), 32, axis=2)
            .reshape(NG, P, S).transpose(1, 0, 2).reshape(P, NG * S)
        ).astype(bf)
        in_maps.append(
            {
                "h_nat": np.ascontiguousarray(h_nat[lo:hi]),
                "h8t": np.ascontiguousarray(h8t[lo:hi]),
                "w1t8": w1t8,
                "w2t8": w2t8,
                "htt_bf": htt,
                "u_col": u_col,
                "w1b_col": w1b_col,
                "maskrep": mrep,
            }
        )
    return in_maps


def kernel(h, h_mask, ht, w1_w, w1_b, u_w):
    nc = build()
    in_maps = _prep_core_inputs(h, h_mask, ht, w1_w, w1_b, u_w)
    res = bass_utils.run_bass_kernel_spmd(
        nc,
        in_maps,
        core_ids=list(range(NCORES)),
        trace=bool(int(os.environ.get("KERNEL_TRACE", "0"))),
    )
    _CACHE["last_result"] = res
    out = np.concatenate([r["out"] for r in res.results], axis=0)
    return np.ascontiguousarray(out.astype(np.float32))


# revision 12
# speedup vs baseline: 3.6574x; 1.7016x over previous
"""Trainium2 Bass kernel for nn_Attention_3607772529228 (sparse_attention).

Reference computation (B=64, S=512, T=32, 2H=1024, ATT=512):
    ht_mean = mean(ht, axis=1)                               [B, 2H]
    z       = [h ; ht_mean] @ w1_w.T + w1_b                  [B, S, ATT]
    a       = tanh(z)
    beta    = a @ u_w[0];  beta = where(mask, beta, -1e20)   [B, S]
    alpha   = softmax(beta, axis=1)
    out     = einsum('bs,bsd->bd', alpha, h)                 [B, 2H]

Algebraic simplifications (exact):
  * where(valid, ..., 0) on h_cat / a does not affect the output (invalid
    positions only enter through beta, overwritten with -1e20).
  * The ht_mean half of the big matmul folds into a per-batch bias:
    z = h @ w1.T + (w2 @ ht_mean + w1_b).

Distribution: data-parallel over batch B across 8 cores (8 batches/core).

v3 design (the bf16 baseline is HBM-DMA-bound at ~19MB/core/rep):
  * z path in fp8-e4m3: h_t, w1, w2 shipped fp8 (weights pre-scaled x64 to
    dodge the e4m3 subnormal range; folded back via the tanh activation
    scale). z matmuls use DoubleRow (2 fp8 weights/cell, K=256/pass).
    Final rel err ~0.9e-2 (validated vs 2e-2 budget).
  * h_nat (weighted-sum copy) stays bf16 -- output precision needs it.
  * All big DMAs are host-prepped fully contiguous per partition.
  * Constants (w1/w2/htt/u/mask) + the bias matmuls are hoisted out of
    the rep loop and issued before the h_nat stream.
  * beta lands in a 32x-replicated [128, S] layout (4 batches x 32
    replicas); softmax runs in that layout (no gather DMAs). The -1e20
    mask add is one extra identity-lhsT matmul accumulated into beta's
    PSUM. Exp writes bf16 directly (f32 row-sum via accum_out); the
    1/sum normalization is deferred to the output copy (per-partition
    scale), keeping the beta->alpha->transpose chain short.
  * Per-group tail pipelining: group 0's beta/softmax/wsum PE work is
    emitted between group 1's z matmuls; only group 1's tail is exposed.
  * PE warmup matmuls only on rep 0 (cold path).
"""

import os
from contextlib import ExitStack

import numpy as np
import ml_dtypes

import concourse.bass as bass
import concourse.tile as tile
from concourse import bacc, mybir
from concourse import bass_utils
from concourse.masks import make_identity

BF16 = mybir.dt.bfloat16
F8 = mybir.dt.float8e4
F32 = mybir.dt.float32

B, S, T, H2, ATT = 64, 512, 32, 1024, 512
NCORES = 8
BL = B // NCORES  # 8 batches per core
P = 128
KC = H2 // P  # 8 k-chunks over hidden
KD = KC // 2  # 4 DoubleRow k-pairs
TT = ATT // P  # 4 attention tiles
SC = S // P  # 4 sequence chunks
NH = H2 // 512  # 2 output halves
NG = BL // 4  # batch groups of 4 (PE column-group packing)
WARMUP_MMS = 12
FP8_Z = True  # fp8 z path (h_t/w1/w2) with DoubleRow matmuls
WS = 64.0 if FP8_Z else 1.0  # fp8 weight pre-scale (dodges e4m3 subnormals)

ZDT = F8 if FP8_Z else BF16
NP_F8 = ml_dtypes.float8_e4m3


def _body(tc, reps=1):
    nc = tc.nc
    ctx = tc._ctx

    h_ap = nc.dram_tensor("h_nat", [BL, P, SC * H2], BF16, kind="ExternalInput").ap()
    h8_ap = nc.dram_tensor("h8t", [BL, P, KC * S], ZDT, kind="ExternalInput").ap()
    w1_ap = nc.dram_tensor("w1t8", [P, KC * ATT], ZDT, kind="ExternalInput").ap()
    w2_ap = nc.dram_tensor("w2t8", [P, KC * ATT], ZDT, kind="ExternalInput").ap()
    htt_ap = nc.dram_tensor("htt_bf", [P, KC * BL * T], BF16, kind="ExternalInput").ap()
    u_ap = nc.dram_tensor("u_col", [P, TT * 32], BF16, kind="ExternalInput").ap()
    w1b_ap = nc.dram_tensor("w1b_col", [P, TT], F32, kind="ExternalInput").ap()
    mask_ap = nc.dram_tensor("maskrep", [P, NG * S], BF16, kind="ExternalInput").ap()
    out_ap = nc.dram_tensor("out", [BL, H2], F32, kind="ExternalOutput").ap()

    singles = ctx.enter_context(tc.tile_pool(name="singles", bufs=1))
    hT_pool = ctx.enter_context(tc.tile_pool(name="hT", bufs=3))
    a_pool = ctx.enter_context(tc.tile_pool(name="a", bufs=20))
    rows = ctx.enter_context(tc.tile_pool(name="rows", bufs=4))
    z_psum = ctx.enter_context(tc.tile_pool(name="z_ps", bufs=4, space="PSUM"))
    bias_ws_psum = ctx.enter_context(tc.tile_pool(name="bw_ps", bufs=2, space="PSUM"))
    beta_aT_psum = ctx.enter_context(tc.tile_pool(name="ba_ps", bufs=2, space="PSUM"))

    # ================= prologue: warmup, constants, bias =================
    warm = singles.tile([P, S], BF16)
    nc.vector.memset(warm, 0.0)
    warm_ps = bias_ws_psum.tile([P, S], F32, tag="bw")
    for _ in range(WARMUP_MMS):
        nc.tensor.matmul(warm_ps, lhsT=warm[:, 0:P], rhs=warm, start=True, stop=True)

    # scalar HWDGE queue: htt first (bias path), then the per-rep hT stream
    htt_sb = singles.tile([P, KC, BL * T], BF16)
    nc.scalar.dma_start(out=htt_sb, in_=htt_ap.rearrange("p (k j) -> p k j", k=KC))

    # sync HWDGE queue: weights + small constants, ahead of the h_nat stream
    w1t_sb = singles.tile([P, KC, ATT], ZDT)
    nc.sync.dma_start(out=w1t_sb, in_=w1_ap.rearrange("p (k a) -> p k a", k=KC))
    w2t_sb = singles.tile([P, KC, ATT], ZDT)
    nc.sync.dma_start(out=w2t_sb, in_=w2_ap.rearrange("p (k a) -> p k a", k=KC))
    u_sb = singles.tile([P, TT, 32], BF16)
    nc.sync.dma_start(out=u_sb, in_=u_ap.rearrange("p (t r) -> p t r", t=TT))
    w1b_sb = singles.tile([P, TT], F32)
    nc.sync.dma_start(out=w1b_sb, in_=w1b_ap)
    mask_sb = singles.tile([P, NG, S], BF16)
    nc.sync.dma_start(out=mask_sb, in_=mask_ap.rearrange("p (g s) -> p g s", g=NG))
    ident = singles.tile([P, P], BF16)
    make_identity(nc, ident)

    # ht sum -> (fp8) columns; bias_col[t] = (w2*WS @ ht_sum)/(T*WS) + w1_b
    htm = singles.tile([P, KC, BL], BF16)
    for c in range(KC):
        with nc.allow_low_precision("bf16 sum of 32 bf16 values, fp32 internal"):
            nc.vector.reduce_sum(
                out=htm[:, c, :],
                in_=htt_sb[:, c, :].rearrange("p (b t) -> p b t", b=BL),
                axis=mybir.AxisListType.X,
            )
    if FP8_Z:
        htm_z = singles.tile([P, KC, BL], ZDT)
        nc.vector.tensor_copy(out=htm_z, in_=htm)
    else:
        htm_z = htm
    bias_col = singles.tile([P, TT, BL], F32)
    for t in range(TT):
        b2_ps = bias_ws_psum.tile([P, S], F32, tag="bw")
        for c in range(KC):
            nc.tensor.matmul(
                b2_ps[:, 0:BL],
                lhsT=w2t_sb[:, c, t * P : (t + 1) * P],
                rhs=htm_z[:, c, :],
                start=(c == 0),
                stop=(c == KC - 1),
            )
        nc.vector.tensor_scalar(
            out=bias_col[:, t, :],
            in0=b2_ps[:, 0:BL],
            scalar1=1.0 / (T * WS),
            scalar2=w1b_sb[:, t : t + 1],
            op0=mybir.AluOpType.mult,
            op1=mybir.AluOpType.add,
        )

    # ========================== per-rep body ==========================
    def emit_rep():
        hT_tiles = [None] * BL
        a_tiles = {}
        beta_tiles = {}

        def load_hT(b):
            hT_b = hT_pool.tile([P, KC, S], ZDT, tag="hT")
            nc.scalar.dma_start(
                out=hT_b, in_=h8_ap[b].rearrange("p (k s) -> p k s", k=KC)
            )
            hT_tiles[b] = hT_b

        load_hT(0)
        load_hT(1)

        h_nat = singles.tile([P, BL, SC, H2], BF16)

        def load_hnat(b):
            nc.sync.dma_start(
                out=h_nat[:, b],
                in_=h_ap[b].rearrange("p (sc d) -> p sc d", sc=SC),
            )

        load_hnat(0)

        def emit_beta(g):
            # beta for 4 batches, batch 4g+j on partitions 32j..32j+31 (x32)
            beta_ps = beta_aT_psum.tile([P, S], F32, tag="ba")
            for bb in range(4):
                b = 4 * g + bb
                for t in range(TT):
                    nc.tensor.matmul(
                        beta_ps[32 * bb : 32 * bb + 32, :],
                        lhsT=u_sb[:, t, :],
                        rhs=a_tiles[(b, t)],
                        start=(t == 0),
                        stop=(t == TT - 1),
                        tile_position=(0, 32 * bb),
                    )
            # + mask (0 / -1e20, replicated layout) via identity-lhsT matmul
            nc.tensor.matmul(
                beta_ps,
                lhsT=ident,
                rhs=mask_sb[:, g, :],
                start=False,
                stop=True,
                skip_group_check=True,
            )
            beta_tiles[g] = beta_ps

        def emit_tail(g):
            # softmax over S (free dim), all 4 batches (x32 replicas) at once
            beta_ps = beta_tiles[g]
            negmax = rows.tile([P, 1], F32, tag="negmax")
            nc.vector.reduce_max(
                out=negmax, in_=beta_ps, axis=mybir.AxisListType.X, negate=True
            )
            alpha_bf = rows.tile([P, S], BF16, tag="alpha")
            sumrow = rows.tile([P, 1], F32, tag="sumrow")
            nc.scalar.activation(
                out=alpha_bf,
                in_=beta_ps,
                func=mybir.ActivationFunctionType.Exp,
                bias=negmax[:, 0:1],
                scale=1.0,
                accum_out=sumrow[:, 0:1],
            )
            rinv = rows.tile([P, 1], F32, tag="rinv")
            nc.vector.reciprocal(rinv, sumrow)

            # PE transpose: [128(4bx32r), S] -> per sc [128(s), 128(4bx32r)]
            alpha_sb = rows.tile([P, SC, P], BF16, tag="alphasb")
            for sc in range(SC):
                aT_ps = beta_aT_psum.tile([P, P], BF16, tag="ba")
                nc.tensor.transpose(
                    aT_ps, alpha_bf[:, sc * P : (sc + 1) * P], ident
                )
                nc.vector.tensor_copy(out=alpha_sb[:, sc, :], in_=aT_ps)

            # weighted sum (unnormalized), 4 batches in PE column groups;
            # normalization folds into the output copy as per-partition scale
            for nh in range(NH):
                ws_ps = bias_ws_psum.tile([P, 512], F32, tag="bw")
                for bb in range(4):
                    b = 4 * g + bb
                    for sc in range(SC):
                        nc.tensor.matmul(
                            ws_ps[32 * bb : 32 * bb + 32, :],
                            lhsT=alpha_sb[:, sc, 32 * bb : 32 * bb + 32],
                            rhs=h_nat[:, b, sc, nh * 512 : (nh + 1) * 512],
                            start=(sc == 0),
                            stop=(sc == SC - 1),
                            tile_position=(0, 32 * bb),
                        )
                o_sc = rows.tile([P, 512], F32, tag="orow")
                nc.vector.tensor_scalar_mul(o_sc, ws_ps, rinv[:, 0:1])
                # strided gather: partitions {0,32,64,96} -> out rows
                nc.gpsimd.dma_start(
                    out=out_ap[4 * g : 4 * g + 4, nh * 512 : (nh + 1) * 512],
                    in_=o_sc.rearrange("(b r) s -> b r s", r=32)[:, 0, :],
                )

        def emit_z(b, t):
            z_ps = z_psum.tile([P, S], F32, tag="z")
            hT_b = hT_tiles[b]
            if FP8_Z:
                for kk in range(KD):
                    nc.tensor.matmul(
                        z_ps,
                        lhsT=w1t_sb[:, 2 * kk : 2 * kk + 2, t * P : (t + 1) * P],
                        rhs=hT_b[:, 2 * kk : 2 * kk + 2, :],
                        start=(kk == 0),
                        stop=(kk == KD - 1),
                        perf_mode=mybir.MatmulPerfMode.DoubleRow,
                    )
            else:
                for k in range(KC):
                    nc.tensor.matmul(
                        z_ps,
                        lhsT=w1t_sb[:, k, t * P : (t + 1) * P],
                        rhs=hT_b[:, k, :],
                        start=(k == 0),
                        stop=(k == KC - 1),
                    )
            return z_ps

        for b in range(BL):
            if b + 2 < BL:
                load_hT(b + 2)
            if b + 1 < BL:
                load_hnat(b + 1)
            for t in range(TT):
                z_ps = emit_z(b, t)
                a_t = a_pool.tile([P, S], BF16, tag="a")
                nc.scalar.activation(
                    out=a_t,
                    in_=z_ps,
                    func=mybir.ActivationFunctionType.Tanh,
                    bias=bias_col[:, t, b : b + 1],
                    scale=1.0 / WS,
                )
                a_tiles[(b, t)] = a_t
            if b == 4:
                emit_beta(0)
            elif b == 5:
                emit_tail(0)
        emit_beta(1)
        emit_tail(1)

    for _rep in range(reps):
        emit_rep()


_CACHE = {}


def build(reps=1):
    key = ("nc", reps)
    if key in _CACHE:
        return _CACHE[key]
    nc = bacc.Bacc("TRN2", target_bir_lowering=False, debug=False)
    with tile.TileContext(nc) as tc:
        with ExitStack() as ctx:
            tc._ctx = ctx
            _body(tc, reps=reps)
    nc.compile()
    _CACHE[key] = nc
    return nc


def _prep_core_inputs(h, h_mask, ht, w1_w, w1_b, u_w):
    """Host-side sharding + layout prep. Returns list of 8 per-core dicts."""
    bf = ml_dtypes.bfloat16
    zdt = NP_F8 if FP8_Z else bf
    h = np.asarray(h, dtype=np.float32)
    ht = np.asarray(ht, dtype=np.float32)

    # h_nat[b, p, sc*H2 + d] = h[b, sc*128+p, d]   (contiguous per partition)
    h_nat = np.ascontiguousarray(
        h.reshape(B, SC, P, H2).transpose(0, 2, 1, 3).reshape(B, P, SC * H2)
    ).astype(bf)
    # h8t[b, p, k*S + s] = h[b, s, k*128+p]
    h8t = np.ascontiguousarray(
        h.transpose(0, 2, 1).reshape(B, KC, P, S).transpose(0, 2, 1, 3)
        .reshape(B, P, KC * S)
    ).astype(zdt)

    def prep_w(w):  # [ATT, H2] -> [P, KC*ATT]: w8[p, k*ATT+a] = w[a, k*128+p]
        wt = np.ascontiguousarray(np.asarray(w, dtype=np.float32).T)  # [H2, ATT]
        return np.ascontiguousarray(
            (wt * WS).reshape(KC, P, ATT).transpose(1, 0, 2).reshape(P, KC * ATT)
        ).astype(zdt)

    w1t8 = prep_w(w1_w[:, :H2])
    w2t8 = prep_w(w1_w[:, H2:])

    u_col = np.ascontiguousarray(
        np.repeat(
            np.asarray(u_w[0], dtype=np.float32).reshape(TT, P).T[:, :, None],
            32,
            axis=2,
        ).reshape(P, TT * 32)
    ).astype(bf)
    w1b_col = np.ascontiguousarray(
        np.asarray(w1_b, dtype=np.float32).reshape(TT, P).T
    ).astype(np.float32)

    neg = np.float32(-1e20)
    maskadd = np.where(np.asarray(h_mask) != 0, np.float32(0.0), neg)  # [B, S]

    in_maps = []
    for core in range(NCORES):
        lo, hi = core * BL, (core + 1) * BL
        htc = ht[lo:hi].reshape(BL * T, H2).T  # [H2, BL*T]
        htt = np.ascontiguousarray(
            htc.reshape(KC, P, BL * T).transpose(1, 0, 2).reshape(P, KC * BL * T)
        ).astype(bf)
        # mrep[32*j+r, g*S+s] = maskadd[lo + 4g+j, s]
        mrep = np.ascontiguousarray(
            np.repeat(maskadd[lo:hi].reshape(NG, 4, 1, S), 32, axis=2)
            .reshape(NG, P, S).transpose(1, 0, 2).reshape(P, NG * S)
        ).astype(bf)
        in_maps.append(
            {
                "h_nat": np.ascontiguousarray(h_nat[lo:hi]),
                "h8t": np.ascontiguousarray(h8t[lo:hi]),
                "w1t8": w1t8,
                "w2t8": w2t8,
                "htt_bf": htt,
                "u_col": u_col,
                "w1b_col": w1b_col,
                "maskrep": mrep,
            }
        )
    return in_maps


def kernel(h, h_mask, ht, w1_w, w1_b, u_w):
    nc = build()
    in_maps = _prep_core_inputs(h, h_mask, ht, w1_w, w1_b, u_w)
    res = bass_utils.run_bass_kernel_spmd(
        nc,
        in_maps,
        core_ids=list(range(NCORES)),
        trace=bool(int(os.environ.get("KERNEL_TRACE", "0"))),
    )
    _CACHE["last_result"] = res
    out = np.concatenate([r["out"] for r in res.results], axis=0)
    return np.ascontiguousarray(out.astype(np.float32))
